# revision 33
# baseline (speedup 1.0000x reference)
"""Trainium2 Bass kernel for causal linear-complexity multi-head attention.

Reference computation (per batch n):
    q = softmax((query @ Wq.T) per-head, axis=Dh)
    k = softmax((key  @ Wk.T) per-head, axis=Dh)
    v = key @ Wv.T
    S[t] = sum_{s<=t} k_s^T v_s          (per-head Dh x Dh running state)
    out[t] = q_t @ S[t]

Sharding: 8 cores = 2 batches x 4 head-groups (4 heads of 64 dims each per
core).  The host packs chunk-major bf16 inputs; each core emits its
(L x 256) output slice in natural [t, j] layout plus the raw q-softmax
denominators; the host divides and reassembles (q-normalization is a pure
per-(head, t) post-scale, so it is free on the host).

Device algorithm: chunked linear attention, chunk C=128, all-bf16 matmul
operands (fp32 PSUM accumulation).  Two-deep software pipeline: iteration i
runs stage A (projections + exp + denominators) for chunk i and stage B
(masked intra-chunk attention, inter-chunk term from the running state,
state update) for chunk i-2, so every cross-engine dependency has a full
iteration of slack.  Key structural points:
  - q/k projected in transposed layout [j, t] (one 4D PSUM tile, single
    exp on ACT), v in natural layout [s, j],
  - K=64 per-head matmuls (A^T = ek^T eq, inter-chunk q.S) must read from
    partition base 0 (base-64 bf16 ldweights hangs the device), so the
    upper 64 partitions of the exp tile are shifted down via a SBUF->SBUF
    DMA on the Pool/SWDGE ring (off the critical path),
  - k-softmax denominators ride on v: one DVE multiply against a
    per-(head, s) reciprocal broadcast,
  - causal mask applied as one fused DVE multiply (broadcast mask over the
    4 per-head A^T tiles) during the PSUM->SBUF move,
  - ek transposed to natural layout on the PE (identity-rhs matmul; the
    xbar DMA transpose congests HWDGE and is slower end-to-end),
  - the running state accumulates in fp32 SBUF; stage B snapshots it to
    bf16 for the next chunk's inter-chunk matmul (the final hand-off skips
    the fp32 accumulate and feeds the chunk delta directly).
"""

import threading
from contextlib import ExitStack

import numpy as np

import concourse.mybir as mybir
import concourse.tile as tile
from concourse import bacc
from concourse.bass_utils import run_bass_kernel_spmd

P = 128          # SBUF partitions
D = 1024         # model dim (contraction)
DC = D // P      # d-chunks
J = 256          # per-core output columns (4 heads x 64)
L = 2048         # sequence length
C = 128          # chunk size
NCH = L // C     # chunks
DH = 64          # per-head dim
NHEAD = 4        # heads per core
N_CORES = 8
CW = C + 2 + P + P  # consts width: mask | eones | ident | e2

F32 = mybir.dt.float32
BF16 = mybir.dt.bfloat16
EXP = mybir.ActivationFunctionType.Exp
COPY = mybir.ActivationFunctionType.Copy


def _build_nc():
    nc = bacc.Bacc(trn_type="TRN2", target_bir_lowering=False, num_devices=N_CORES)

    xqk = nc.dram_tensor("xqk", [NCH, P, 2, DC, C], BF16, kind="ExternalInput").ap()
    wq = nc.dram_tensor("wq", [P, DC, J], BF16, kind="ExternalInput").ap()
    wk = nc.dram_tensor("wk", [P, DC, J], BF16, kind="ExternalInput").ap()
    wv = nc.dram_tensor("wv", [P, DC, J], BF16, kind="ExternalInput").ap()
    consts = nc.dram_tensor("consts", [P, CW], BF16, kind="ExternalInput").ap()
    outd = nc.dram_tensor("out", [NCH, P, J], BF16, kind="ExternalOutput").ap()
    outl = nc.dram_tensor("outl", [P, J], BF16, kind="ExternalOutput").ap()
    dqd = nc.dram_tensor("dq", [2, 2, NCH, C], BF16, kind="ExternalOutput").ap()

    xqk_r = xqk.rearrange("c p q d t -> p c q d t")
    out_r = outd.rearrange("c p j -> p c j")

    with tile.TileContext(nc) as tc, ExitStack() as ctx:
        ctx.enter_context(
            nc.allow_low_precision(reason="bf16 operands; fp32 accumulation")
        )
        cpool = ctx.enter_context(tc.tile_pool(name="consts", bufs=1))
        xpool = ctx.enter_context(tc.tile_pool(name="xin", bufs=4))
        spool = ctx.enter_context(tc.tile_pool(name="sb", bufs=3))
        opool = ctx.enter_context(tc.tile_pool(name="ob", bufs=2))
        pqkp = ctx.enter_context(tc.tile_pool(name="pqk", bufs=2, space="PSUM"))
        pvp = ctx.enter_context(tc.tile_pool(name="pv", bufs=1, space="PSUM"))
        patp = ctx.enter_context(tc.tile_pool(name="pat", bufs=1, space="PSUM"))
        poutp = ctx.enter_context(tc.tile_pool(name="po", bufs=1, space="PSUM"))
        pmisc = ctx.enter_context(tc.tile_pool(name="pm", bufs=1, space="PSUM"))
        psp = ctx.enter_context(tc.tile_pool(name="ps", bufs=1, space="PSUM"))

        # ---- initial DMAs: stagger halves so PE starts early ----
        HD = DC // 2
        wq_sb = cpool.tile([P, DC, J], BF16, tag="wq_sb")
        wk_sb = cpool.tile([P, DC, J], BF16, tag="wk_sb")
        wv_sb = cpool.tile([P, DC, J], BF16, tag="wv_sb")
        x_t = {}

        def load_x(c):
            x_t[c] = xpool.tile([P, 2, DC, C], BF16, tag="x", name=f"x{c}")
            nc.sync.dma_start(x_t[c][:], xqk_r[:, c])

        x_t[0] = xpool.tile([P, 2, DC, C], BF16, tag="x", name="x0")
        x_t[1] = xpool.tile([P, 2, DC, C], BF16, tag="x", name="x1")
        nc.sync.dma_start(wq_sb[:, 0:HD, :], wq[:, 0:HD, :])
        nc.sync.dma_start(x_t[0][:, :, 0:HD, :], xqk_r[:, 0, :, 0:HD, :])
        nc.sync.dma_start(wq_sb[:, HD:DC, :], wq[:, HD:DC, :])
        nc.sync.dma_start(x_t[0][:, :, HD:DC, :], xqk_r[:, 0, :, HD:DC, :])
        nc.sync.dma_start(wk_sb[:, 0:HD, :], wk[:, 0:HD, :])
        nc.sync.dma_start(wk_sb[:, HD:DC, :], wk[:, HD:DC, :])
        nc.sync.dma_start(wv_sb[:, 0:HD, :], wv[:, 0:HD, :])
        nc.sync.dma_start(wv_sb[:, HD:DC, :], wv[:, HD:DC, :])
        nc.sync.dma_start(x_t[1][:, :, 0:HD, :], xqk_r[:, 1, :, 0:HD, :])
        nc.sync.dma_start(x_t[1][:, :, HD:DC, :], xqk_r[:, 1, :, HD:DC, :])
        consts_sb = cpool.tile([P, CW], BF16, tag="consts_sb")
        nc.sync.dma_start(consts_sb[:], consts[:])
        load_x(2)

        mask_sb = consts_sb[:, 0:C]
        eones_sb = consts_sb[:, C:C + 2]
        ident_sb = consts_sb[:, C + 2:C + 2 + P]

        # per-chunk q-softmax denominators, staged for one final DMA;
        # the q normalization itself happens on the host
        dq_all = cpool.tile([2, 2, NCH, C], BF16, tag="dq_all")

        # per-chunk tiles carried across the pipeline boundary
        eqk_e = {}      # exp(q)/exp(k), T layout  [P, 2(qk), 2(jt), C] bf16
        eqk_lo = {}     # partitions 64-127 of eqk_e shifted to 0-63 (DMA)
        v_sb = {}       # v natural (k-denominator folded in)  [P, J] bf16
        at_m = {}       # masked A^T per head  [P, NHEAD, C] bf16
        ekn_sb = {}     # k natural  [P, 2(jt), C(j)] bf16
        S_sb = {}       # bf16 running state after chunk c (bf16 accumulation)

        def at_block(c):
            """Intra-chunk logits A^T = ek^T eq per head, masked."""
            pat = patp.tile([P, NHEAD, C], F32, tag="pat")
            for h in range(NHEAD):
                jt, half = h // 2, h % 2
                src_t = eqk_e[c] if half == 0 else eqk_lo[c]
                nc.tensor.matmul(
                    pat[:, h, :],
                    src_t[0:DH, 1, jt, :],
                    src_t[0:DH, 0, jt, :],
                    start=True,
                    stop=True,
                )
            at_m[c] = spool.tile([P, NHEAD, C], BF16, tag="at_m", name="at_m")
            nc.vector.tensor_mul(
                at_m[c][:], pat[:],
                mask_sb[:, None, :].broadcast_to([P, NHEAD, C]),
            )

        def stage_b(c):
            """Attention epilogue for chunk c (inter+intra out, state)."""
            pout = poutp.tile([P, J], F32, tag="pout")
            for h in range(NHEAD):
                jt, half = h // 2, h % 2
                hs = slice(h * DH, (h + 1) * DH)
                nc.tensor.matmul(
                    pout[:, hs],
                    at_m[c][:, h, :],
                    v_sb[c][:, hs],
                    start=True,
                    stop=(c == 0),
                )
                if c > 0:
                    src_t = eqk_e[c] if half == 0 else eqk_lo[c]
                    nc.tensor.matmul(
                        pout[:, hs],
                        src_t[0:DH, 0, jt, :],
                        S_sb[c - 1][0:DH, h, :],
                        start=False,
                        stop=True,
                    )
            oc = opool.tile([P, J], BF16, tag="oc")
            nc.scalar.activation(oc[:], pout[:], COPY)
            if c == NCH - 1:
                nc.sync.dma_start(outl[:], oc[:])
            else:
                nc.sync.dma_start(out_r[:, c, :], oc[:])
            if c < NCH - 1:
                pds = psp.tile([DH, NHEAD, DH], F32, tag="pds")
                for h in range(NHEAD):
                    jt, half = h // 2, h % 2
                    hs = slice(h * DH, (h + 1) * DH)
                    nc.tensor.matmul(
                        pds[:, h, :],
                        ekn_sb[c][:, jt, DH * half:DH * half + DH],
                        v_sb[c][:, hs],
                        start=True,
                        stop=True,
                    )
                S_sb[c] = spool.tile([DH, NHEAD, DH], BF16, tag="S_sb",
                                     name="S_sb")
                if c == 0:
                    nc.vector.tensor_copy(S_sb[c][:], pds[:])
                else:
                    nc.vector.tensor_add(S_sb[c][:], S_sb[c - 1][:], pds[:])
            # drop refs consumed by this stage
            del eqk_e[c], v_sb[c], at_m[c], eqk_lo[c]
            ekn_sb.pop(c, None)
            S_sb.pop(c - 3, None)

        for i in range(NCH):
            if i + 3 < NCH:
                load_x(i + 3)

            # ---- stage A(i): projections ----
            pqk = pqkp.tile([P, 2, 2, C], F32, tag="pqk")
            for qk, w_sb in ((0, wq_sb), (1, wk_sb)):
                for jt in range(2):
                    for dc in range(DC):
                        nc.tensor.matmul(
                            pqk[:, qk, jt, :],
                            w_sb[:, dc, jt * P:(jt + 1) * P],
                            x_t[i][:, qk, dc, :],
                            start=(dc == 0),
                            stop=(dc == DC - 1),
                        )
            eqk_e[i] = spool.tile([P, 2, 2, C], BF16, tag="eqk_e", name="eqk_e")
            nc.scalar.activation(eqk_e[i][:], pqk[:], EXP)

            # ---- transpose ek(i-1); AT(i-2) — fills the exp(i) latency ----
            if 1 <= i < NCH:
                if i - 1 < NCH - 1:
                    ptr = pmisc.tile([P, 2, P], F32, tag="ptr")
                    for jt in range(2):
                        nc.tensor.matmul(
                            ptr[:, jt, :], eqk_e[i - 1][:, 1, jt, :],
                            ident_sb[:],
                            start=True, stop=True,
                        )
                    ekn_sb[i - 1] = spool.tile([P, 2, C], BF16, tag="ekn_sb",
                                               name="ekn_sb")
                    nc.scalar.activation(ekn_sb[i - 1][:], ptr[:], COPY)
            if i >= 2:
                at_block(i - 2)
            # shift partitions 64-127 down to 0-63 for the K=64 matmuls
            eqk_lo[i] = spool.tile([DH, 2, 2, C], BF16, tag="eqk_lo",
                                   name="eqk_lo")
            if i == NCH - 1:
                nc.gpsimd.dma_start(eqk_lo[i][:], eqk_e[i][DH:P, :, :, :])

            pv = pvp.tile([P, J], F32, tag="pv")
            for dc in range(DC):
                nc.tensor.matmul(
                    pv[:],
                    x_t[i][:, 1, dc, :],
                    wv_sb[:, dc, :],
                    start=(dc == 0),
                    stop=(dc == DC - 1),
                )

            # ---- k denominators [s, head] -> v scale ----
            pmix = pmisc.tile([P, 2 * C + 4], F32, tag="pmix")
            pdk = pmix[:, 2 * C:2 * C + 4].rearrange("p (a b) -> p a b", b=2)
            for jt in range(2):
                nc.tensor.matmul(
                    pdk[:, jt, :],
                    eqk_e[i][:, 1, jt, :],
                    eones_sb[:],
                    start=True,
                    stop=True,
                )
            rk = spool.tile([P, 2, 2], F32, tag="rk")
            nc.vector.reciprocal(rk[:], pdk)
            v_sb[i] = spool.tile([P, J], BF16, tag="v_sb", name="v_sb")
            nc.vector.tensor_mul(
                v_sb[i][:].rearrange("p (a b) -> p a b", b=DH),
                pv[:].rearrange("p (a b) -> p a b", b=DH),
                rk[:].rearrange("p a b -> p (a b)")[:, :, None].broadcast_to(
                    [P, 4, DH]),
            )

            # ---- q denominators [head, jt, t] ----
            pdq = pmix[0:2, 0:2 * C].rearrange("p (a b) -> p a b", b=C)
            nc.tensor.matmul(
                pdq, eones_sb[:], eqk_e[i][:, 0, :, :], start=True, stop=True
            )
            nc.scalar.activation(dq_all[0:2, :, i, :], pdq, COPY)
            if i == NCH - 1:
                nc.gpsimd.dma_start(dqd[:], dq_all[:])

            # ---- stage B(i-2) ----
            if i >= 2:
                stage_b(i - 2)
            if i < NCH - 1:
                # late shift issue: keeps the Pool SWDGE generation from
                # delaying the state snapshot copy
                nc.gpsimd.dma_start(eqk_lo[i][:], eqk_e[i][DH:P, :, :, :])
            if i == NCH - 1:
                at_block(i - 1)
                at_block(i)
                stage_b(i - 1)

        stage_b(NCH - 1)

    nc.finalize()
    return nc


def _host_inputs(query, key, Wq, Wk, Wv):
    """Build the 8 per-core input maps (host-side layout prep)."""
    import ml_dtypes
    bf = ml_dtypes.bfloat16

    s = np.arange(P)[:, None]
    t = np.arange(C)[None, :]
    consts = np.zeros((P, CW), np.float32)
    consts[:, 0:C] = (s <= t).astype(np.float32)
    eones = np.zeros((P, 2), np.float32)
    eones[:DH, 0] = 1.0
    eones[DH:, 1] = 1.0
    consts[:, C:C + 2] = eones
    consts[:, C + 2:C + 2 + P] = np.eye(P, dtype=np.float32)
    e2 = np.zeros((2, P), np.float32)
    e2[0, :DH] = 1.0
    e2[1, DH:] = 1.0
    consts[0:2, C + 2 + P:CW] = e2
    consts = consts.astype(bf)

    per_batch = {}
    for n in range(2):
        xq = query[n].T.reshape(DC, P, NCH, C).transpose(2, 1, 0, 3)
        xk = key[n].T.reshape(DC, P, NCH, C).transpose(2, 1, 0, 3)
        per_batch[n] = np.ascontiguousarray(
            np.stack([xq, xk], axis=2)).astype(bf)

    w_parts = {}
    for g in range(4):
        cols = slice(g * J, (g + 1) * J)
        w_parts[g] = tuple(
            np.ascontiguousarray(
                W[cols, :].T.reshape(DC, P, J).transpose(1, 0, 2)
            ).astype(bf)
            for W in (Wq, Wk, Wv)
        )

    in_maps = []
    for core in range(N_CORES):
        n, g = core // 4, core % 4
        wqp, wkp, wvp = w_parts[g]
        in_maps.append({
            "xqk": per_batch[n],
            "wq": wqp,
            "wk": wkp,
            "wv": wvp,
            "consts": consts,
        })
    return in_maps


_NC_LOCK = threading.Lock()
_NC_CACHE = {}


def _get_nc():
    with _NC_LOCK:
        if "nc" not in _NC_CACHE:
            _NC_CACHE["nc"] = _build_nc()
        return _NC_CACHE["nc"]


def kernel(query, key, Wq, Wk, Wv, _trace=False, _trace_kwargs=None):
    query = np.asarray(query)
    key = np.asarray(key)
    Wq = np.asarray(Wq)
    Wk = np.asarray(Wk)
    Wv = np.asarray(Wv)

    nc = _get_nc()
    in_maps = _host_inputs(query, key, Wq, Wk, Wv)
    res = run_bass_kernel_spmd(
        nc, in_maps, core_ids=list(range(N_CORES)),
        trace=_trace, **(_trace_kwargs or {}),
    )

    out = np.empty((2, L, D), np.float32)
    for core, r in enumerate(res.results):
        n, g = core // 4, core % 4
        raw = r["out"].astype(np.float32).reshape(L, J)
        raw[L - C:L, :] = r["outl"].astype(np.float32)
        dq = r["dq"].astype(np.float32)          # [h2, jt, NCH, C]
        div = dq.transpose(2, 3, 1, 0).reshape(L, 4)  # cols = head
        out[n, :, g * J:(g + 1) * J] = raw / np.repeat(div, DH, axis=1)
    if _trace:
        kernel.last_results = res
    return out


# revision 34
# speedup vs baseline: 1.0007x; 1.0007x over previous
"""Trainium2 Bass kernel for causal linear-complexity multi-head attention.

Reference computation (per batch n):
    q = softmax((query @ Wq.T) per-head, axis=Dh)
    k = softmax((key  @ Wk.T) per-head, axis=Dh)
    v = key @ Wv.T
    S[t] = sum_{s<=t} k_s^T v_s          (per-head Dh x Dh running state)
    out[t] = q_t @ S[t]

Sharding: 8 cores = 2 batches x 4 head-groups (4 heads of 64 dims each per
core).  The host packs chunk-major bf16 inputs; each core emits its
(L x 256) output slice in natural [t, j] layout plus the raw q-softmax
denominators; the host divides and reassembles (q-normalization is a pure
per-(head, t) post-scale, so it is free on the host).

Device algorithm: chunked linear attention, chunk C=128, all-bf16 matmul
operands (fp32 PSUM accumulation).  Two-deep software pipeline: iteration i
runs stage A (projections + exp + denominators) for chunk i and stage B
(masked intra-chunk attention, inter-chunk term from the running state,
state update) for chunk i-2, so every cross-engine dependency has a full
iteration of slack.  Key structural points:
  - q/k projected in transposed layout [j, t] (one 4D PSUM tile, single
    exp on ACT), v in natural layout [s, j],
  - K=64 per-head matmuls (A^T = ek^T eq, inter-chunk q.S) must read from
    partition base 0 (base-64 bf16 ldweights hangs the device), so the
    upper 64 partitions of the exp tile are shifted down via a SBUF->SBUF
    DMA on the Pool/SWDGE ring (off the critical path),
  - k-softmax denominators ride on v: one DVE multiply against a
    per-(head, s) reciprocal broadcast,
  - causal mask applied as one fused DVE multiply (broadcast mask over the
    4 per-head A^T tiles) during the PSUM->SBUF move,
  - ek transposed to natural layout on the PE (identity-rhs matmul; the
    xbar DMA transpose congests HWDGE and is slower end-to-end),
  - the running state accumulates in fp32 SBUF; stage B snapshots it to
    bf16 for the next chunk's inter-chunk matmul (the final hand-off skips
    the fp32 accumulate and feeds the chunk delta directly).
"""

import threading
from contextlib import ExitStack

import numpy as np

import concourse.mybir as mybir
import concourse.tile as tile
from concourse import bacc
from concourse.bass_utils import run_bass_kernel_spmd

P = 128          # SBUF partitions
D = 1024         # model dim (contraction)
DC = D // P      # d-chunks
J = 256          # per-core output columns (4 heads x 64)
L = 2048         # sequence length
C = 128          # chunk size
NCH = L // C     # chunks
DH = 64          # per-head dim
NHEAD = 4        # heads per core
N_CORES = 8
CW = C + 2 + P + P  # consts width: mask | eones | ident | e2

F32 = mybir.dt.float32
BF16 = mybir.dt.bfloat16
EXP = mybir.ActivationFunctionType.Exp
COPY = mybir.ActivationFunctionType.Copy


def _build_nc():
    nc = bacc.Bacc(trn_type="TRN2", target_bir_lowering=False, num_devices=N_CORES)

    xqk = nc.dram_tensor("xqk", [NCH, P, 2, DC, C], BF16, kind="ExternalInput").ap()
    wq = nc.dram_tensor("wq", [P, DC, J], BF16, kind="ExternalInput").ap()
    wk = nc.dram_tensor("wk", [P, DC, J], BF16, kind="ExternalInput").ap()
    wv = nc.dram_tensor("wv", [P, DC, J], BF16, kind="ExternalInput").ap()
    consts = nc.dram_tensor("consts", [P, CW], BF16, kind="ExternalInput").ap()
    outd = nc.dram_tensor("out", [NCH, P, J], BF16, kind="ExternalOutput").ap()
    outl = nc.dram_tensor("outl", [P, J], BF16, kind="ExternalOutput").ap()
    dqd = nc.dram_tensor("dq", [2, 2, NCH, C], BF16, kind="ExternalOutput").ap()

    xqk_r = xqk.rearrange("c p q d t -> p c q d t")
    out_r = outd.rearrange("c p j -> p c j")

    with tile.TileContext(nc) as tc, ExitStack() as ctx:
        ctx.enter_context(
            nc.allow_low_precision(reason="bf16 operands; fp32 accumulation")
        )
        cpool = ctx.enter_context(tc.tile_pool(name="consts", bufs=1))
        xpool = ctx.enter_context(tc.tile_pool(name="xin", bufs=4))
        spool = ctx.enter_context(tc.tile_pool(name="sb", bufs=3))
        opool = ctx.enter_context(tc.tile_pool(name="ob", bufs=2))
        pqkp = ctx.enter_context(tc.tile_pool(name="pqk", bufs=2, space="PSUM"))
        pvp = ctx.enter_context(tc.tile_pool(name="pv", bufs=1, space="PSUM"))
        patp = ctx.enter_context(tc.tile_pool(name="pat", bufs=1, space="PSUM"))
        poutp = ctx.enter_context(tc.tile_pool(name="po", bufs=1, space="PSUM"))
        pmisc = ctx.enter_context(tc.tile_pool(name="pm", bufs=1, space="PSUM"))
        psp = ctx.enter_context(tc.tile_pool(name="ps", bufs=1, space="PSUM"))

        # ---- initial DMAs: stagger halves so PE starts early ----
        HD = DC // 2
        wq_sb = cpool.tile([P, DC, J], BF16, tag="wq_sb")
        wk_sb = cpool.tile([P, DC, J], BF16, tag="wk_sb")
        wv_sb = cpool.tile([P, DC, J], BF16, tag="wv_sb")
        x_t = {}

        def load_x(c):
            x_t[c] = xpool.tile([P, 2, DC, C], BF16, tag="x", name=f"x{c}")
            nc.sync.dma_start(x_t[c][:], xqk_r[:, c])

        x_t[0] = xpool.tile([P, 2, DC, C], BF16, tag="x", name="x0")
        x_t[1] = xpool.tile([P, 2, DC, C], BF16, tag="x", name="x1")
        nc.sync.dma_start(wq_sb[:, 0:HD, :], wq[:, 0:HD, :])
        nc.sync.dma_start(x_t[0][:, :, 0:HD, :], xqk_r[:, 0, :, 0:HD, :])
        nc.sync.dma_start(wq_sb[:, HD:DC, :], wq[:, HD:DC, :])
        nc.sync.dma_start(x_t[0][:, :, HD:DC, :], xqk_r[:, 0, :, HD:DC, :])
        nc.sync.dma_start(wk_sb[:, 0:HD, :], wk[:, 0:HD, :])
        nc.sync.dma_start(wk_sb[:, HD:DC, :], wk[:, HD:DC, :])
        nc.sync.dma_start(wv_sb[:, 0:HD, :], wv[:, 0:HD, :])
        nc.sync.dma_start(wv_sb[:, HD:DC, :], wv[:, HD:DC, :])
        nc.sync.dma_start(x_t[1][:, :, 0:HD, :], xqk_r[:, 1, :, 0:HD, :])
        nc.sync.dma_start(x_t[1][:, :, HD:DC, :], xqk_r[:, 1, :, HD:DC, :])
        consts_sb = cpool.tile([P, CW], BF16, tag="consts_sb")
        nc.sync.dma_start(consts_sb[:], consts[:])
        load_x(2)

        mask_sb = consts_sb[:, 0:C]
        eones_sb = consts_sb[:, C:C + 2]
        ident_sb = consts_sb[:, C + 2:C + 2 + P]

        # per-chunk q-softmax denominators, staged for one final DMA;
        # the q normalization itself happens on the host
        dq_all = cpool.tile([2, 2, NCH, C], BF16, tag="dq_all")

        # per-chunk tiles carried across the pipeline boundary
        eqk_e = {}      # exp(q)/exp(k), T layout  [P, 2(qk), 2(jt), C] bf16
        eqk_lo = {}     # partitions 64-127 of eqk_e shifted to 0-63 (DMA)
        v_sb = {}       # v natural (k-denominator folded in)  [P, J] bf16
        at_m = {}       # masked A^T per head  [P, NHEAD, C] bf16
        ekn_sb = {}     # k natural  [P, 2(jt), C(j)] bf16
        S_sb = {}       # bf16 running state after chunk c (bf16 accumulation)

        def at_block(c):
            """Intra-chunk logits A^T = ek^T eq per head, masked."""
            pat = patp.tile([P, NHEAD, C], F32, tag="pat")
            for h in range(NHEAD):
                jt, half = h // 2, h % 2
                src_t = eqk_e[c] if half == 0 else eqk_lo[c]
                nc.tensor.matmul(
                    pat[:, h, :],
                    src_t[0:DH, 1, jt, :],
                    src_t[0:DH, 0, jt, :],
                    start=True,
                    stop=True,
                )
            at_m[c] = spool.tile([P, NHEAD, C], BF16, tag="at_m", name="at_m")
            nc.vector.tensor_mul(
                at_m[c][:], pat[:],
                mask_sb[:, None, :].broadcast_to([P, NHEAD, C]),
            )

        def stage_b(c):
            """Attention epilogue for chunk c (inter+intra out, state)."""
            pout = poutp.tile([P, J], F32, tag="pout")
            for h in range(NHEAD):
                jt, half = h // 2, h % 2
                hs = slice(h * DH, (h + 1) * DH)
                nc.tensor.matmul(
                    pout[:, hs],
                    at_m[c][:, h, :],
                    v_sb[c][:, hs],
                    start=True,
                    stop=(c == 0),
                )
                if c > 0:
                    src_t = eqk_e[c] if half == 0 else eqk_lo[c]
                    nc.tensor.matmul(
                        pout[:, hs],
                        src_t[0:DH, 0, jt, :],
                        S_sb[c - 1][0:DH, h, :],
                        start=False,
                        stop=True,
                    )
            oc = opool.tile([P, J], BF16, tag="oc")
            nc.scalar.activation(oc[:], pout[:], COPY)
            if c == NCH - 1:
                nc.sync.dma_start(outl[:], oc[:])
            else:
                nc.sync.dma_start(out_r[:, c, :], oc[:])
            if c < NCH - 1:
                pds = psp.tile([DH, NHEAD, DH], F32, tag="pds")
                for h in range(NHEAD):
                    jt, half = h // 2, h % 2
                    hs = slice(h * DH, (h + 1) * DH)
                    nc.tensor.matmul(
                        pds[:, h, :],
                        ekn_sb[c][:, jt, DH * half:DH * half + DH],
                        v_sb[c][:, hs],
                        start=True,
                        stop=True,
                    )
                S_sb[c] = spool.tile([DH, NHEAD, DH], BF16, tag="S_sb",
                                     name="S_sb")
                if c == 0:
                    nc.vector.tensor_copy(S_sb[c][:], pds[:])
                else:
                    nc.vector.tensor_add(S_sb[c][:], S_sb[c - 1][:], pds[:])
            # drop refs consumed by this stage
            del eqk_e[c], v_sb[c], at_m[c], eqk_lo[c]
            ekn_sb.pop(c, None)
            S_sb.pop(c - 3, None)

        for i in range(NCH):
            if i + 3 < NCH:
                load_x(i + 3)

            # ---- stage A(i): projections ----
            pqk = pqkp.tile([P, 2, 2, C], F32, tag="pqk")
            for qk, w_sb in ((0, wq_sb), (1, wk_sb)):
                for jt in range(2):
                    for dc in range(DC):
                        nc.tensor.matmul(
                            pqk[:, qk, jt, :],
                            w_sb[:, dc, jt * P:(jt + 1) * P],
                            x_t[i][:, qk, dc, :],
                            start=(dc == 0),
                            stop=(dc == DC - 1),
                        )
            eqk_e[i] = spool.tile([P, 2, 2, C], BF16, tag="eqk_e", name="eqk_e")
            nc.scalar.activation(eqk_e[i][:], pqk[:], EXP)

            # ---- transpose ek(i-1); AT(i-2) — fills the exp(i) latency ----
            if 1 <= i < NCH:
                if i - 1 < NCH - 1:
                    ptr = pmisc.tile([P, 2, P], F32, tag="ptr")
                    for jt in range(2):
                        nc.tensor.matmul(
                            ptr[:, jt, :], eqk_e[i - 1][:, 1, jt, :],
                            ident_sb[:],
                            start=True, stop=True,
                        )
                    ekn_sb[i - 1] = spool.tile([P, 2, C], BF16, tag="ekn_sb",
                                               name="ekn_sb")
                    nc.scalar.activation(ekn_sb[i - 1][:], ptr[:], COPY)
            if i >= 2:
                at_block(i - 2)
            # shift partitions 64-127 down to 0-63 for the K=64 matmuls
            eqk_lo[i] = spool.tile([DH, 2, 2, C], BF16, tag="eqk_lo",
                                   name="eqk_lo")
            if i == NCH - 1:
                nc.gpsimd.dma_start(eqk_lo[i][:], eqk_e[i][DH:P, :, :, :])

            pv = pvp.tile([P, J], F32, tag="pv")
            for dc in range(DC):
                nc.tensor.matmul(
                    pv[:],
                    x_t[i][:, 1, dc, :],
                    wv_sb[:, dc, :],
                    start=(dc == 0),
                    stop=(dc == DC - 1),
                )

            # ---- k denominators [s, head] -> v scale ----
            pmix = pmisc.tile([P, 2 * C + 4], F32, tag="pmix")
            pdk = pmix[:, 2 * C:2 * C + 4].rearrange("p (a b) -> p a b", b=2)
            for jt in range(2):
                nc.tensor.matmul(
                    pdk[:, jt, :],
                    eqk_e[i][:, 1, jt, :],
                    eones_sb[:],
                    start=True,
                    stop=True,
                )
            rk = spool.tile([P, 2, 2], F32, tag="rk")
            nc.vector.reciprocal(rk[:], pdk)
            v_sb[i] = spool.tile([P, J], BF16, tag="v_sb", name="v_sb")
            nc.vector.tensor_mul(
                v_sb[i][:].rearrange("p (a b) -> p a b", b=DH),
                pv[:].rearrange("p (a b) -> p a b", b=DH),
                rk[:].rearrange("p a b -> p (a b)")[:, :, None].broadcast_to(
                    [P, 4, DH]),
            )

            # ---- q denominators [head, jt, t] ----
            pdq = pmix[0:2, 0:2 * C].rearrange("p (a b) -> p a b", b=C)
            nc.tensor.matmul(
                pdq, eones_sb[:], eqk_e[i][:, 0, :, :], start=True, stop=True
            )
            nc.scalar.activation(dq_all[0:2, :, i, :], pdq, COPY)
            if i == NCH - 1:
                nc.sync.dma_start(dqd[:], dq_all[:])

            # ---- stage B(i-2) ----
            if i >= 2:
                stage_b(i - 2)
            if i < NCH - 1:
                # late shift issue: keeps the Pool SWDGE generation from
                # delaying the state snapshot copy
                nc.gpsimd.dma_start(eqk_lo[i][:], eqk_e[i][DH:P, :, :, :])
            if i == NCH - 1:
                at_block(i - 1)
                at_block(i)
                stage_b(i - 1)

        stage_b(NCH - 1)

    nc.finalize()
    return nc


def _host_inputs(query, key, Wq, Wk, Wv):
    """Build the 8 per-core input maps (host-side layout prep)."""
    import ml_dtypes
    bf = ml_dtypes.bfloat16

    s = np.arange(P)[:, None]
    t = np.arange(C)[None, :]
    consts = np.zeros((P, CW), np.float32)
    consts[:, 0:C] = (s <= t).astype(np.float32)
    eones = np.zeros((P, 2), np.float32)
    eones[:DH, 0] = 1.0
    eones[DH:, 1] = 1.0
    consts[:, C:C + 2] = eones
    consts[:, C + 2:C + 2 + P] = np.eye(P, dtype=np.float32)
    e2 = np.zeros((2, P), np.float32)
    e2[0, :DH] = 1.0
    e2[1, DH:] = 1.0
    consts[0:2, C + 2 + P:CW] = e2
    consts = consts.astype(bf)

    per_batch = {}
    for n in range(2):
        xq = query[n].T.reshape(DC, P, NCH, C).transpose(2, 1, 0, 3)
        xk = key[n].T.reshape(DC, P, NCH, C).transpose(2, 1, 0, 3)
        per_batch[n] = np.ascontiguousarray(
            np.stack([xq, xk], axis=2)).astype(bf)

    w_parts = {}
    for g in range(4):
        cols = slice(g * J, (g + 1) * J)
        w_parts[g] = tuple(
            np.ascontiguousarray(
                W[cols, :].T.reshape(DC, P, J).transpose(1, 0, 2)
            ).astype(bf)
            for W in (Wq, Wk, Wv)
        )

    in_maps = []
    for core in range(N_CORES):
        n, g = core // 4, core % 4
        wqp, wkp, wvp = w_parts[g]
        in_maps.append({
            "xqk": per_batch[n],
            "wq": wqp,
            "wk": wkp,
            "wv": wvp,
            "consts": consts,
        })
    return in_maps


_NC_LOCK = threading.Lock()
_NC_CACHE = {}


def _get_nc():
    with _NC_LOCK:
        if "nc" not in _NC_CACHE:
            _NC_CACHE["nc"] = _build_nc()
        return _NC_CACHE["nc"]


def kernel(query, key, Wq, Wk, Wv, _trace=False, _trace_kwargs=None):
    query = np.asarray(query)
    key = np.asarray(key)
    Wq = np.asarray(Wq)
    Wk = np.asarray(Wk)
    Wv = np.asarray(Wv)

    nc = _get_nc()
    in_maps = _host_inputs(query, key, Wq, Wk, Wv)
    res = run_bass_kernel_spmd(
        nc, in_maps, core_ids=list(range(N_CORES)),
        trace=_trace, **(_trace_kwargs or {}),
    )

    out = np.empty((2, L, D), np.float32)
    for core, r in enumerate(res.results):
        n, g = core // 4, core % 4
        raw = r["out"].astype(np.float32).reshape(L, J)
        raw[L - C:L, :] = r["outl"].astype(np.float32)
        dq = r["dq"].astype(np.float32)          # [h2, jt, NCH, C]
        div = dq.transpose(2, 3, 1, 0).reshape(L, 4)  # cols = head
        out[n, :, g * J:(g + 1) * J] = raw / np.repeat(div, DH, axis=1)
    if _trace:
        kernel.last_results = res
    return out


# revision 35
# speedup vs baseline: 1.0035x; 1.0027x over previous
"""Trainium2 Bass kernel for causal linear-complexity multi-head attention.

Reference computation (per batch n):
    q = softmax((query @ Wq.T) per-head, axis=Dh)
    k = softmax((key  @ Wk.T) per-head, axis=Dh)
    v = key @ Wv.T
    S[t] = sum_{s<=t} k_s^T v_s          (per-head Dh x Dh running state)
    out[t] = q_t @ S[t]

Sharding: 8 cores = 2 batches x 4 head-groups (4 heads of 64 dims each per
core).  The host packs chunk-major bf16 inputs; each core emits its
(L x 256) output slice in natural [t, j] layout plus the raw q-softmax
denominators; the host divides and reassembles (q-normalization is a pure
per-(head, t) post-scale, so it is free on the host).

Device algorithm: chunked linear attention, chunk C=128, all-bf16 matmul
operands (fp32 PSUM accumulation).  Two-deep software pipeline: iteration i
runs stage A (projections + exp + denominators) for chunk i and stage B
(masked intra-chunk attention, inter-chunk term from the running state,
state update) for chunk i-2, so every cross-engine dependency has a full
iteration of slack.  Key structural points:
  - q/k projected in transposed layout [j, t] (one 4D PSUM tile, single
    exp on ACT), v in natural layout [s, j],
  - K=64 per-head matmuls (A^T = ek^T eq, inter-chunk q.S) must read from
    partition base 0 (base-64 bf16 ldweights hangs the device), so the
    upper 64 partitions of the exp tile are shifted down via a SBUF->SBUF
    DMA on the Pool/SWDGE ring (off the critical path),
  - k-softmax denominators ride on v: one DVE multiply against a
    per-(head, s) reciprocal broadcast,
  - causal mask applied as one fused DVE multiply (broadcast mask over the
    4 per-head A^T tiles) during the PSUM->SBUF move,
  - ek transposed to natural layout on the PE (identity-rhs matmul; the
    xbar DMA transpose congests HWDGE and is slower end-to-end),
  - the running state accumulates in fp32 SBUF; stage B snapshots it to
    bf16 for the next chunk's inter-chunk matmul (the final hand-off skips
    the fp32 accumulate and feeds the chunk delta directly).
"""

import threading
from contextlib import ExitStack

import numpy as np

import concourse.mybir as mybir
import concourse.tile as tile
from concourse import bacc
from concourse.bass_utils import run_bass_kernel_spmd

P = 128          # SBUF partitions
D = 1024         # model dim (contraction)
DC = D // P      # d-chunks
J = 256          # per-core output columns (4 heads x 64)
L = 2048         # sequence length
C = 128          # chunk size
NCH = L // C     # chunks
DH = 64          # per-head dim
NHEAD = 4        # heads per core
N_CORES = 8
CW = C + 2 + P + P  # consts width: mask | eones | ident | e2

F32 = mybir.dt.float32
BF16 = mybir.dt.bfloat16
EXP = mybir.ActivationFunctionType.Exp
COPY = mybir.ActivationFunctionType.Copy


def _build_nc():
    nc = bacc.Bacc(trn_type="TRN2", target_bir_lowering=False, num_devices=N_CORES)

    xqk = nc.dram_tensor("xqk", [NCH, P, 2, DC, C], BF16, kind="ExternalInput").ap()
    wq = nc.dram_tensor("wq", [P, DC, J], BF16, kind="ExternalInput").ap()
    wk = nc.dram_tensor("wk", [P, DC, J], BF16, kind="ExternalInput").ap()
    wv = nc.dram_tensor("wv", [P, DC, J], BF16, kind="ExternalInput").ap()
    consts = nc.dram_tensor("consts", [P, CW], BF16, kind="ExternalInput").ap()
    outd = nc.dram_tensor("out", [NCH, P, J], BF16, kind="ExternalOutput").ap()
    outl = nc.dram_tensor("outl", [P, J], BF16, kind="ExternalOutput").ap()
    dqd = nc.dram_tensor("dq", [2, 2, NCH, C], BF16, kind="ExternalOutput").ap()

    xqk_r = xqk.rearrange("c p q d t -> p c q d t")
    out_r = outd.rearrange("c p j -> p c j")

    with tile.TileContext(nc) as tc, ExitStack() as ctx:
        ctx.enter_context(
            nc.allow_low_precision(reason="bf16 operands; fp32 accumulation")
        )
        cpool = ctx.enter_context(tc.tile_pool(name="consts", bufs=1))
        xpool = ctx.enter_context(tc.tile_pool(name="xin", bufs=4))
        spool = ctx.enter_context(tc.tile_pool(name="sb", bufs=3))
        opool = ctx.enter_context(tc.tile_pool(name="ob", bufs=2))
        pqkp = ctx.enter_context(tc.tile_pool(name="pqk", bufs=2, space="PSUM"))
        pvp = ctx.enter_context(tc.tile_pool(name="pv", bufs=1, space="PSUM"))
        patp = ctx.enter_context(tc.tile_pool(name="pat", bufs=1, space="PSUM"))
        poutp = ctx.enter_context(tc.tile_pool(name="po", bufs=1, space="PSUM"))
        pmisc = ctx.enter_context(tc.tile_pool(name="pm", bufs=1, space="PSUM"))
        psp = ctx.enter_context(tc.tile_pool(name="ps", bufs=1, space="PSUM"))

        # ---- initial DMAs: stagger halves so PE starts early ----
        HD = DC // 2
        wq_sb = cpool.tile([P, DC, J], BF16, tag="wq_sb")
        wk_sb = cpool.tile([P, DC, J], BF16, tag="wk_sb")
        wv_sb = cpool.tile([P, DC, J], BF16, tag="wv_sb")
        x_t = {}

        def load_x(c):
            x_t[c] = xpool.tile([P, 2, DC, C], BF16, tag="x", name=f"x{c}")
            nc.sync.dma_start(x_t[c][:], xqk_r[:, c])

        x_t[0] = xpool.tile([P, 2, DC, C], BF16, tag="x", name="x0")
        x_t[1] = xpool.tile([P, 2, DC, C], BF16, tag="x", name="x1")
        nc.sync.dma_start(wq_sb[:, 0:HD, :], wq[:, 0:HD, :])
        nc.sync.dma_start(x_t[0][:, :, 0:HD, :], xqk_r[:, 0, :, 0:HD, :])
        nc.sync.dma_start(wq_sb[:, HD:DC, :], wq[:, HD:DC, :])
        nc.sync.dma_start(x_t[0][:, :, HD:DC, :], xqk_r[:, 0, :, HD:DC, :])
        nc.sync.dma_start(wk_sb[:, 0:HD, :], wk[:, 0:HD, :])
        nc.sync.dma_start(wk_sb[:, HD:DC, :], wk[:, HD:DC, :])
        nc.sync.dma_start(wv_sb[:, 0:HD, :], wv[:, 0:HD, :])
        nc.sync.dma_start(wv_sb[:, HD:DC, :], wv[:, HD:DC, :])
        nc.sync.dma_start(x_t[1][:, :, 0:HD, :], xqk_r[:, 1, :, 0:HD, :])
        nc.sync.dma_start(x_t[1][:, :, HD:DC, :], xqk_r[:, 1, :, HD:DC, :])
        consts_sb = cpool.tile([P, CW], BF16, tag="consts_sb")
        nc.sync.dma_start(consts_sb[:], consts[:])
        load_x(2)

        mask_sb = consts_sb[:, 0:C]
        eones_sb = consts_sb[:, C:C + 2]
        ident_sb = consts_sb[:, C + 2:C + 2 + P]

        # per-chunk q-softmax denominators, staged for one final DMA;
        # the q normalization itself happens on the host
        dq_all = cpool.tile([2, 2, NCH, C], BF16, tag="dq_all")

        # zero-padded exp(q) for the last chunk: lets its AT matmuls run
        # without waiting for the partition-shift DMA (zero rows make the
        # K=128 contraction select the head implicitly)
        eqz15 = cpool.tile([P, 2, 2 * C], BF16, tag="eqz15")
        nc.vector.memset(eqz15[:].bitcast(F32), 0.0)

        # per-chunk tiles carried across the pipeline boundary
        eqk_e = {}      # exp(q)/exp(k), T layout  [P, 2(qk), 2(jt), C] bf16
        eqk_lo = {}     # partitions 64-127 of eqk_e shifted to 0-63 (DMA)
        v_sb = {}       # v natural (k-denominator folded in)  [P, J] bf16
        at_m = {}       # masked A^T per head  [P, NHEAD, C] bf16
        ekn_sb = {}     # k natural  [P, 2(jt), C(j)] bf16
        S_sb = {}       # bf16 running state after chunk c (bf16 accumulation)

        def at_block(c):
            """Intra-chunk logits A^T = ek^T eq per head, masked."""
            pat = patp.tile([P, NHEAD, C], F32, tag="pat")
            at_m[c] = spool.tile([P, NHEAD, C], BF16, tag="at_m", name="at_m")
            if c == NCH - 1:
                for jt in range(2):
                    nc.tensor.matmul(
                        pat[:, 2 * jt:2 * jt + 2, :],
                        eqk_e[c][:, 1, jt, :],
                        eqz15[:, jt, :],
                        start=True,
                        stop=True,
                    )
                    nc.vector.tensor_mul(
                        at_m[c][:, 2 * jt:2 * jt + 2, :],
                        pat[:, 2 * jt:2 * jt + 2, :],
                        mask_sb[:, None, :].broadcast_to([P, 2, C]),
                    )
                return
            for h in range(NHEAD):
                jt, half = h // 2, h % 2
                src_t = eqk_e[c] if half == 0 else eqk_lo[c]
                nc.tensor.matmul(
                    pat[:, h, :],
                    src_t[0:DH, 1, jt, :],
                    src_t[0:DH, 0, jt, :],
                    start=True,
                    stop=True,
                )
            nc.vector.tensor_mul(
                at_m[c][:], pat[:],
                mask_sb[:, None, :].broadcast_to([P, NHEAD, C]),
            )

        def stage_b(c):
            """Attention epilogue for chunk c (inter+intra out, state)."""
            pout = poutp.tile([P, J], F32, tag="pout")
            for h in range(NHEAD):
                jt, half = h // 2, h % 2
                hs = slice(h * DH, (h + 1) * DH)
                nc.tensor.matmul(
                    pout[:, hs],
                    at_m[c][:, h, :],
                    v_sb[c][:, hs],
                    start=True,
                    stop=(c == 0),
                )
                if c > 0:
                    src_t = eqk_e[c] if half == 0 else eqk_lo[c]
                    nc.tensor.matmul(
                        pout[:, hs],
                        src_t[0:DH, 0, jt, :],
                        S_sb[c - 1][0:DH, h, :],
                        start=False,
                        stop=True,
                    )
            oc = opool.tile([P, J], BF16, tag="oc")
            nc.scalar.activation(oc[:], pout[:], COPY)
            if c == NCH - 1:
                nc.sync.dma_start(outl[:], oc[:])
            else:
                nc.sync.dma_start(out_r[:, c, :], oc[:])
            if c < NCH - 1:
                pds = psp.tile([DH, NHEAD, DH], F32, tag="pds")
                for h in range(NHEAD):
                    jt, half = h // 2, h % 2
                    hs = slice(h * DH, (h + 1) * DH)
                    nc.tensor.matmul(
                        pds[:, h, :],
                        ekn_sb[c][:, jt, DH * half:DH * half + DH],
                        v_sb[c][:, hs],
                        start=True,
                        stop=True,
                    )
                S_sb[c] = spool.tile([DH, NHEAD, DH], BF16, tag="S_sb",
                                     name="S_sb")
                if c == 0:
                    nc.vector.tensor_copy(S_sb[c][:], pds[:])
                else:
                    nc.vector.tensor_add(S_sb[c][:], S_sb[c - 1][:], pds[:])
            # drop refs consumed by this stage
            del eqk_e[c], v_sb[c], at_m[c], eqk_lo[c]
            ekn_sb.pop(c, None)
            S_sb.pop(c - 3, None)

        for i in range(NCH):
            if i + 3 < NCH:
                load_x(i + 3)

            # ---- stage A(i): projections ----
            pqk = pqkp.tile([P, 2, 2, C], F32, tag="pqk")
            for qk, w_sb in ((0, wq_sb), (1, wk_sb)):
                for jt in range(2):
                    for dc in range(DC):
                        nc.tensor.matmul(
                            pqk[:, qk, jt, :],
                            w_sb[:, dc, jt * P:(jt + 1) * P],
                            x_t[i][:, qk, dc, :],
                            start=(dc == 0),
                            stop=(dc == DC - 1),
                        )
            eqk_e[i] = spool.tile([P, 2, 2, C], BF16, tag="eqk_e", name="eqk_e")
            nc.scalar.activation(eqk_e[i][:], pqk[:], EXP)
            if i == NCH - 1:
                nc.scalar.activation(
                    eqz15[0:DH, :, 0:C], pqk[0:DH, 0, :, :], EXP)
                nc.scalar.activation(
                    eqz15[DH:P, :, C:2 * C], pqk[DH:P, 0, :, :], EXP)

            # ---- transpose ek(i-1); AT(i-2) — fills the exp(i) latency ----
            if 1 <= i < NCH:
                if i - 1 < NCH - 1:
                    ptr = pmisc.tile([P, 2, P], F32, tag="ptr")
                    for jt in range(2):
                        nc.tensor.matmul(
                            ptr[:, jt, :], eqk_e[i - 1][:, 1, jt, :],
                            ident_sb[:],
                            start=True, stop=True,
                        )
                    ekn_sb[i - 1] = spool.tile([P, 2, C], BF16, tag="ekn_sb",
                                               name="ekn_sb")
                    nc.scalar.activation(ekn_sb[i - 1][:], ptr[:], COPY)
            if i >= 2:
                at_block(i - 2)
            # shift partitions 64-127 down to 0-63 for the K=64 matmuls
            eqk_lo[i] = spool.tile([DH, 2, 2, C], BF16, tag="eqk_lo",
                                   name="eqk_lo")
            if i == NCH - 1:
                nc.gpsimd.dma_start(eqk_lo[i][:], eqk_e[i][DH:P, :, :, :])

            pv = pvp.tile([P, J], F32, tag="pv")
            for dc in range(DC):
                nc.tensor.matmul(
                    pv[:],
                    x_t[i][:, 1, dc, :],
                    wv_sb[:, dc, :],
                    start=(dc == 0),
                    stop=(dc == DC - 1),
                )

            # ---- k denominators [s, head] -> v scale ----
            pmix = pmisc.tile([P, 2 * C + 4], F32, tag="pmix")
            pdk = pmix[:, 2 * C:2 * C + 4].rearrange("p (a b) -> p a b", b=2)
            for jt in range(2):
                nc.tensor.matmul(
                    pdk[:, jt, :],
                    eqk_e[i][:, 1, jt, :],
                    eones_sb[:],
                    start=True,
                    stop=True,
                )
            rk = spool.tile([P, 2, 2], F32, tag="rk")
            nc.vector.reciprocal(rk[:], pdk)
            v_sb[i] = spool.tile([P, J], BF16, tag="v_sb", name="v_sb")
            nc.vector.tensor_mul(
                v_sb[i][:].rearrange("p (a b) -> p a b", b=DH),
                pv[:].rearrange("p (a b) -> p a b", b=DH),
                rk[:].rearrange("p a b -> p (a b)")[:, :, None].broadcast_to(
                    [P, 4, DH]),
            )

            # ---- q denominators [head, jt, t] ----
            pdq = pmix[0:2, 0:2 * C].rearrange("p (a b) -> p a b", b=C)
            nc.tensor.matmul(
                pdq, eones_sb[:], eqk_e[i][:, 0, :, :], start=True, stop=True
            )
            nc.scalar.activation(dq_all[0:2, :, i, :], pdq, COPY)
            if i == NCH - 1:
                nc.sync.dma_start(dqd[:], dq_all[:])

            # ---- stage B(i-2) ----
            if i >= 2:
                stage_b(i - 2)
            if i < NCH - 1:
                # late shift issue: keeps the Pool SWDGE generation from
                # delaying the state snapshot copy
                nc.gpsimd.dma_start(eqk_lo[i][:], eqk_e[i][DH:P, :, :, :])
            if i == NCH - 1:
                at_block(i - 1)
                at_block(i)
                stage_b(i - 1)

        stage_b(NCH - 1)

    nc.finalize()
    return nc


def _host_inputs(query, key, Wq, Wk, Wv):
    """Build the 8 per-core input maps (host-side layout prep)."""
    import ml_dtypes
    bf = ml_dtypes.bfloat16

    s = np.arange(P)[:, None]
    t = np.arange(C)[None, :]
    consts = np.zeros((P, CW), np.float32)
    consts[:, 0:C] = (s <= t).astype(np.float32)
    eones = np.zeros((P, 2), np.float32)
    eones[:DH, 0] = 1.0
    eones[DH:, 1] = 1.0
    consts[:, C:C + 2] = eones
    consts[:, C + 2:C + 2 + P] = np.eye(P, dtype=np.float32)
    e2 = np.zeros((2, P), np.float32)
    e2[0, :DH] = 1.0
    e2[1, DH:] = 1.0
    consts[0:2, C + 2 + P:CW] = e2
    consts = consts.astype(bf)

    per_batch = {}
    for n in range(2):
        xq = query[n].T.reshape(DC, P, NCH, C).transpose(2, 1, 0, 3)
        xk = key[n].T.reshape(DC, P, NCH, C).transpose(2, 1, 0, 3)
        per_batch[n] = np.ascontiguousarray(
            np.stack([xq, xk], axis=2)).astype(bf)

    w_parts = {}
    for g in range(4):
        cols = slice(g * J, (g + 1) * J)
        w_parts[g] = tuple(
            np.ascontiguousarray(
                W[cols, :].T.reshape(DC, P, J).transpose(1, 0, 2)
            ).astype(bf)
            for W in (Wq, Wk, Wv)
        )

    in_maps = []
    for core in range(N_CORES):
        n, g = core // 4, core % 4
        wqp, wkp, wvp = w_parts[g]
        in_maps.append({
            "xqk": per_batch[n],
            "wq": wqp,
            "wk": wkp,
            "wv": wvp,
            "consts": consts,
        })
    return in_maps


_NC_LOCK = threading.Lock()
_NC_CACHE = {}


def _get_nc():
    with _NC_LOCK:
        if "nc" not in _NC_CACHE:
            _NC_CACHE["nc"] = _build_nc()
        return _NC_CACHE["nc"]


def kernel(query, key, Wq, Wk, Wv, _trace=False, _trace_kwargs=None):
    query = np.asarray(query)
    key = np.asarray(key)
    Wq = np.asarray(Wq)
    Wk = np.asarray(Wk)
    Wv = np.asarray(Wv)

    nc = _get_nc()
    in_maps = _host_inputs(query, key, Wq, Wk, Wv)
    res = run_bass_kernel_spmd(
        nc, in_maps, core_ids=list(range(N_CORES)),
        trace=_trace, **(_trace_kwargs or {}),
    )

    out = np.empty((2, L, D), np.float32)
    for core, r in enumerate(res.results):
        n, g = core // 4, core % 4
        raw = r["out"].astype(np.float32).reshape(L, J)
        raw[L - C:L, :] = r["outl"].astype(np.float32)
        dq = r["dq"].astype(np.float32)          # [h2, jt, NCH, C]
        div = dq.transpose(2, 3, 1, 0).reshape(L, 4)  # cols = head
        out[n, :, g * J:(g + 1) * J] = raw / np.repeat(div, DH, axis=1)
    if _trace:
        kernel.last_results = res
    return out
